# revision 30
# baseline (speedup 1.0000x reference)
"""BinaryTreeLSTM Trainium2 kernel (B=32 trees, 512 leaves, dim 1024).

Sharding: data-parallel over trees -- 4 trees per core on 8 NeuronCores,
gate weights replicated.

Per-core design:
  - Activations are kept feature-major [feat_chunk(128), tree, node_col].
  - Leaves are pre-permuted on the host by 9-bit bit-reversal, which makes
    the two children of output node j sit at columns (j, j+n) at *every*
    level -> all child reads are contiguous block slices (no strided APs).
  - Matmul operands fp16 (the 20 MiB of transposed gate weights stay
    SBUF-resident the whole kernel; their load is emitted after the leaf
    phase so the leaf's own DMAs win the queues at startup), PSUM
    accumulation, elementwise math and the cell state c in fp32, h stored
    fp16 between levels.
  - The leaf level skips the forget-gate matmul (child state is zero).
  - Levels down to n=32 nodes/tree round-trip c (fp32) / h (fp16) through
    DRAM scratch; the last five tiny levels (n<=16) keep state in SBUF to
    avoid DMA latency on the critical path.
"""

import sys

if "/opt/trn_rl_repo" not in sys.path:
    sys.path.insert(0, "/opt/trn_rl_repo")

import numpy as np

import concourse.bass as bass
import concourse.tile as tile
from concourse import bacc, mybir
from concourse.bass_utils import run_bass_kernel_spmd

F16 = mybir.dt.float16
F32 = mybir.dt.float32
AF = mybir.ActivationFunctionType

NCORES = 8
B = 32                  # trees total
T = B // NCORES         # trees per core
NL = 512                # leaves per tree
D = 1024                # IN_DIM == MEM
KX = D // 128           # 8 k-chunks for the leaf matmul
KH = 2 * D // 128       # 16 k-chunks for internal matmuls
MF = D // 128           # 8 feature chunks per gate
N_SBUF_TAIL = 16        # levels with <= this many nodes/tree keep c/h in SBUF

_CACHE = {}


def _bitrev(nbits):
    n = 1 << nbits
    p = np.zeros(n, np.int64)
    for i in range(n):
        r = 0
        for b in range(nbits):
            if i >> b & 1:
                r |= 1 << (nbits - 1 - b)
        p[i] = r
    return p


def _build_program(reps=1):
    """reps>1 wraps the compute body in a hardware For_i loop -- used only
    for timing (axon dispatch overhead is ~80 ms per launch, so the kernel
    must be repeated on-device to be measurable via wall-clock slope)."""
    nc = bacc.Bacc("TRN2", target_bir_lowering=False, debug=False,
                   num_devices=NCORES)
    xT = nc.dram_tensor("xT", [KX, 128, T, NL], F16, kind="ExternalInput")
    wxT = nc.dram_tensor("wxT", [3, MF, 128, KX, 128], F16,
                         kind="ExternalInput")
    whT = nc.dram_tensor("whT", [KH, 128, 5 * D], F16, kind="ExternalInput")
    biou = nc.dram_tensor("biou", [128, 3 * MF], F32, kind="ExternalInput")
    bf = nc.dram_tensor("bf", [128, MF], F32, kind="ExternalInput")
    outh = nc.dram_tensor("outh", [MF, 128, T], F32, kind="ExternalOutput")

    with tile.TileContext(nc) as tc:
        with tc.tile_pool(name="consts", bufs=1) as consts, \
             tc.tile_pool(name="whp", bufs=1) as whp, \
             tc.tile_pool(name="stream", bufs=1) as stream, \
             tc.tile_pool(name="evac", bufs=2) as evac, \
             tc.tile_pool(name="dram", bufs=2, space="DRAM") as dram, \
             tc.tile_pool(name="ps", bufs=1, space="PSUM") as ps:

            biou_sb = consts.tile([128, 3 * MF], F32)
            nc.sync.dma_start(out=biou_sb, in_=biou.ap())
            bf_sb = consts.tile([128, MF], F32)
            nc.sync.dma_start(out=bf_sb, in_=bf.ap())

            # resident gate weights: 16 k-chunks x 5120 cols fp16 (160 KiB/p)
            wh_sb = whp.tile([128, KH, 5 * D], F16)

            def load_wh():
                for k in range(KH):
                    nc.sync.dma_start(out=wh_sb[:, k], in_=whT.ap()[k])

            def gates_to_outputs(psl, m, cl_ap, cr_ap, c_dst, h_dst, mode,
                                 small=False):
                """Turn psum gate pre-activations into c/h and store them.

                mode: "dram" -> write c/h via evac tiles + DMA to c_dst/h_dst
                      "sbuf" -> write h (fp16) / c (fp32) straight into the
                                SBUF slices c_dst/h_dst
                      "root" -> h fp32 -> outh[m]
                DVE may read at most one PSUM operand per instruction, so the
                i and o gates are evacuated to SBUF by their activations
                while u/fl/fr stay in PSUM.
                """
                shape = list(psl["i"].shape)
                # tail levels use tiny dedicated tags for deeper pipelining
                gsb_tag, gsb_bufs = ("gsb", 2)
                c_tag, c_bufs = ("c", 2)
                i_sb = evac.tile(shape, F32, tag=gsb_tag, bufs=gsb_bufs,
                                 name="i_sb")
                nc.scalar.activation(out=i_sb, in_=psl["i"], func=AF.Sigmoid,
                                     bias=biou_sb[:, m:m + 1], scale=1.0)
                nc.scalar.activation(out=psl["u"], in_=psl["u"], func=AF.Tanh,
                                     bias=biou_sb[:, 2 * MF + m:2 * MF + m + 1],
                                     scale=1.0)
                c_t = evac.tile(shape, F32, tag=c_tag, bufs=c_bufs, name="c_t")
                nc.vector.tensor_mul(c_t, i_sb, psl["u"])
                if cl_ap is not None:
                    nc.scalar.activation(out=psl["fl"], in_=psl["fl"],
                                         func=AF.Sigmoid,
                                         bias=bf_sb[:, m:m + 1], scale=1.0)
                    nc.vector.tensor_mul(psl["fl"], psl["fl"], cl_ap)
                    nc.vector.tensor_add(c_t, c_t, psl["fl"])
                    nc.scalar.activation(out=psl["fr"], in_=psl["fr"],
                                         func=AF.Sigmoid,
                                         bias=bf_sb[:, m:m + 1], scale=1.0)
                    nc.vector.tensor_mul(psl["fr"], psl["fr"], cr_ap)
                    c_final = c_dst if mode == "sbuf" else c_t
                    nc.vector.tensor_add(c_final, c_t, psl["fr"])
                else:
                    if mode == "sbuf":
                        nc.vector.tensor_copy(c_dst, c_t)
                        c_final = c_dst
                    else:
                        c_final = c_t
                o_sb = evac.tile(shape, F32, tag=gsb_tag, bufs=gsb_bufs,
                                 name="o_sb")
                nc.scalar.activation(out=o_sb, in_=psl["o"], func=AF.Sigmoid,
                                     bias=biou_sb[:, MF + m:MF + m + 1],
                                     scale=1.0)
                th = ps.tile(shape, F32, tag="th", bufs=2, name="th")
                nc.scalar.activation(out=th, in_=c_final, func=AF.Tanh)
                if mode == "root":
                    h_t = evac.tile(shape, F32, tag="hroot", name="h_t")
                    nc.vector.tensor_mul(h_t, o_sb, th)
                    nc.sync.dma_start(out=outh.ap()[m], in_=h_t[:, :, 0])
                elif mode == "sbuf":
                    nc.vector.tensor_mul(h_dst, o_sb, th)
                else:
                    h_t = evac.tile(shape, F16, tag="h", name="h_t")
                    assert not small
                    nc.vector.tensor_mul(h_t, o_sb, th)
                    nc.sync.dma_start(out=h_dst, in_=h_t)
                    nc.sync.dma_start(out=c_dst, in_=c_t)

            def emit_leaf(wh_interleave=False):
                hA = dram.tile([MF, 128, T, NL], F16, tag="h", name="hA")
                cA = dram.tile([MF, 128, T, NL], F32, tag="c", name="cA")
                ncc = 128
                nchunks = NL // ncc
                for ci in range(nchunks):
                    cs = slice(ci * ncc, (ci + 1) * ncc)
                    # alternate between the (leaf-idle) hl/hr slots for
                    # double buffering at zero extra SBUF cost
                    x_t = stream.tile([128, KX, T, ncc], F16,
                                      tag=("hl" if ci % 2 == 0 else "hr"),
                                      name="x_t")
                    for k in range(KX):
                        nc.sync.dma_start(out=x_t[:, k],
                                          in_=xT.ap()[k, :, :, cs])
                    for m in range(MF):
                        psl = {}
                        for g, gt in enumerate(("i", "o", "u")):
                            wxg = stream.tile([128, KX, 128], F16, tag="wx",
                                              bufs=3, name="wxg")
                            nc.sync.dma_start(out=wxg, in_=wxT.ap()[g, m])
                            pt = ps.tile([128, T, ncc], F32, tag=gt,
                                         bufs=(2 if gt == "o" else 1),
                                         name=f"ps_{gt}")
                            for k in range(KX):
                                nc.tensor.matmul(pt, lhsT=wxg[:, k],
                                                 rhs=x_t[:, k],
                                                 start=(k == 0),
                                                 stop=(k == KX - 1))
                            psl[gt] = pt
                        gates_to_outputs(psl, m, None, None,
                                         cA[m, :, :, cs], hA[m, :, :, cs],
                                         mode="dram")
                    if wh_interleave:
                        # spread the resident-weight load across the leaf
                        # phase so it is done when the first level needs it
                        kper = KH // nchunks
                        for k in range(ci * kper, (ci + 1) * kper):
                            nc.sync.dma_start(out=wh_sb[:, k],
                                              in_=whT.ap()[k])
                return hA, cA

            def emit_internal(hA, cA):
                n = NL
                ht_prev = ct_prev = None        # SBUF tail state
                while n > 1:
                    n //= 2                     # output nodes per tree
                    root = n == 1
                    sbuf_out = n <= N_SBUF_TAIL
                    sbuf_in = ht_prev is not None
                    ht_cur = ct_cur = hB = cB = None
                    if sbuf_out and not root:
                        ht_cur = evac.tile([128, MF, T, n], F16, tag="ht",
                                           name="ht_cur")
                        ct_cur = evac.tile([128, MF, T, n], F32, tag="ct",
                                           name="ct_cur")
                    elif not root:
                        hB = dram.tile([MF, 128, T, n], F16, tag="h",
                                       name="hB")
                        cB = dram.tile([MF, 128, T, n], F32, tag="c",
                                       name="cB")
                    ncc = min(n, 128)
                    for ci in range(n // ncc):
                        j0 = ci * ncc
                        if not sbuf_in:
                            hl_t = stream.tile([128, KH // 2, T, ncc], F16,
                                               tag="hl", name="hl_t")
                            hr_t = stream.tile([128, KH // 2, T, ncc], F16,
                                               tag="hr", name="hr_t")
                            for k in range(KH // 2):
                                nc.sync.dma_start(
                                    out=hl_t[:, k],
                                    in_=hA[k, :, :, j0:j0 + ncc])
                                nc.sync.dma_start(
                                    out=hr_t[:, k],
                                    in_=hA[k, :, :, n + j0:n + j0 + ncc])
                        for m in range(MF):
                            if sbuf_in:
                                cl_ap = ct_prev[:, m, :, 0:n]
                                cr_ap = ct_prev[:, m, :, n:2 * n]
                            else:
                                cl_t = stream.tile([128, T, ncc], F32,
                                                   tag="cl", bufs=2,
                                                   name="cl_t")
                                nc.sync.dma_start(
                                    out=cl_t, in_=cA[m, :, :, j0:j0 + ncc])
                                cr_t = stream.tile([128, T, ncc], F32,
                                                   tag="cr", bufs=2,
                                                   name="cr_t")
                                nc.sync.dma_start(
                                    out=cr_t,
                                    in_=cA[m, :, :, n + j0:n + j0 + ncc])
                                cl_ap, cr_ap = cl_t, cr_t
                            psl = {}
                            small = n <= N_SBUF_TAIL
                            for g, gt in enumerate(
                                    ("i", "o", "u", "fl", "fr")):
                                pt = ps.tile([128, T, ncc], F32, tag=gt,
                                             bufs=(2 if gt == "o" else 1),
                                             name=f"ps_{gt}")
                                for k in range(KH):
                                    kk = k % (KH // 2)
                                    if sbuf_in:
                                        rhs = (ht_prev[:, kk, :, 0:n]
                                               if k < KH // 2 else
                                               ht_prev[:, kk, :, n:2 * n])
                                    else:
                                        rhs = (hl_t[:, kk] if k < KH // 2
                                               else hr_t[:, kk])
                                    nc.tensor.matmul(
                                        pt,
                                        lhsT=wh_sb[:, k,
                                                   g * D + m * 128:
                                                   g * D + (m + 1) * 128],
                                        rhs=rhs,
                                        start=(k == 0), stop=(k == KH - 1))
                                psl[gt] = pt
                            if root:
                                gates_to_outputs(psl, m, cl_ap, cr_ap,
                                                 None, None, mode="root",
                                                 small=small)
                            elif sbuf_out:
                                gates_to_outputs(psl, m, cl_ap, cr_ap,
                                                 ct_cur[:, m], ht_cur[:, m],
                                                 mode="sbuf", small=small)
                            else:
                                gates_to_outputs(
                                    psl, m, cl_ap, cr_ap,
                                    cB[m, :, :, j0:j0 + ncc],
                                    hB[m, :, :, j0:j0 + ncc], mode="dram")
                    if not root:
                        if sbuf_out:
                            ht_prev, ct_prev = ht_cur, ct_cur
                        else:
                            hA, cA = hB, cB

            if reps == 1:
                hA, cA = emit_leaf(wh_interleave=True)
                emit_internal(hA, cA)
            else:
                load_wh()
                with tc.For_i(0, reps, 1):
                    hA, cA = emit_leaf()
                    emit_internal(hA, cA)

    nc.compile()
    return nc


def _get_nc(reps=1):
    key = f"nc{reps}"
    if key not in _CACHE:
        _CACHE[key] = _build_program(reps)
    return _CACHE[key]


def _prep_inputs(inputs, w_fioux, b_fioux, w_iouh, w_fh):
    """Host-side prep: permute/transpose/cast, build one in_map per core."""
    perm = _bitrev(9)
    # weights / biases are replicated across cores
    # [g, m, p, k, d]: per-partition-contiguous (k, d) for 2 KiB DMA bursts
    wxT = np.ascontiguousarray(
        w_fioux[D:].reshape(3, MF, 128, KX, 128).transpose(0, 1, 4, 3, 2)
    ).astype(np.float16)
    whT = np.ascontiguousarray(
        np.concatenate([w_iouh, w_fh], axis=0).T).astype(np.float16) \
        .reshape(KH, 128, 5 * D)
    biou = np.ascontiguousarray(
        b_fioux[D:].astype(np.float32).reshape(3 * MF, 128).T)
    bf = np.ascontiguousarray(
        b_fioux[:D].astype(np.float32).reshape(MF, 128).T)
    in_maps = []
    for c in range(NCORES):
        xc = inputs[c * T:(c + 1) * T][:, perm, :]        # [T, NL, D]
        xT = np.ascontiguousarray(xc.transpose(2, 0, 1)).astype(np.float16) \
            .reshape(KX, 128, T, NL)
        in_maps.append({"xT": xT, "wxT": wxT, "whT": whT,
                        "biou": biou, "bf": bf})
    return in_maps


def _assemble(results):
    out = np.zeros((B, D), np.float32)
    for c in range(NCORES):
        oh = results[c]["outh"].reshape(D, T)             # [feat, tree]
        out[c * T:(c + 1) * T] = oh.T
    return out


def kernel(inputs, w_fioux, b_fioux, w_iouh, w_fh):
    inputs = np.asarray(inputs, np.float32)
    w_fioux = np.asarray(w_fioux, np.float32)
    b_fioux = np.asarray(b_fioux, np.float32)
    w_iouh = np.asarray(w_iouh, np.float32)
    w_fh = np.asarray(w_fh, np.float32)
    nc = _get_nc()
    in_maps = _prep_inputs(inputs, w_fioux, b_fioux, w_iouh, w_fh)
    res = run_bass_kernel_spmd(nc, in_maps, core_ids=list(range(NCORES)))
    return _assemble(res.results)


# ---------------------------------------------------------------------------
# benchmarking helper (not used by the grader): builds the jitted SPMD
# callable once so repeated executions can be timed without re-lowering.
def _bench(inputs, w_fioux, b_fioux, w_iouh, w_fh, iters=20, reps=1):
    import time

    import jax
    from jax.experimental.shard_map import shard_map
    from jax.sharding import Mesh, PartitionSpec

    from concourse import bass2jax

    nc = _get_nc(reps)
    in_maps = _prep_inputs(np.asarray(inputs, np.float32),
                           np.asarray(w_fioux, np.float32),
                           np.asarray(b_fioux, np.float32),
                           np.asarray(w_iouh, np.float32),
                           np.asarray(w_fh, np.float32))
    bass2jax.install_neuronx_cc_hook()

    partition_name = (nc.partition_id_tensor.name
                      if nc.partition_id_tensor else None)
    in_names, out_names, out_avals, zero_outs = [], [], [], []
    for alloc in nc.m.functions[0].allocations:
        if not isinstance(alloc, mybir.MemoryLocationSet):
            continue
        name = alloc.memorylocations[0].name
        if alloc.kind == "ExternalInput":
            if name != partition_name:
                in_names.append(name)
        elif alloc.kind == "ExternalOutput":
            out_names.append(name)
            shape = tuple(alloc.tensor_shape)
            dtype = mybir.dt.np(alloc.dtype)
            out_avals.append(jax.core.ShapedArray(shape, dtype))
            zero_outs.append(np.zeros(shape, dtype))
    n_params = len(in_names)
    n_outs = len(out_avals)
    all_in_names = in_names + out_names + \
        ([partition_name] if partition_name else [])

    def _body(*args):
        operands = list(args)
        if partition_name is not None:
            operands.append(bass2jax.partition_id_tensor())
        outs = bass2jax._bass_exec_p.bind(
            *operands,
            out_avals=tuple(out_avals),
            in_names=tuple(all_in_names),
            out_names=tuple(out_names),
            lowering_input_output_aliases=(),
            sim_require_finite=True,
            sim_require_nnan=True,
            nc=nc,
        )
        return tuple(outs)

    devices = jax.devices()[:NCORES]
    mesh = Mesh(np.asarray(devices), ("core",))
    in_specs = (PartitionSpec("core"),) * (n_params + n_outs)
    out_specs = (PartitionSpec("core"),) * n_outs
    donate = tuple(range(n_params, n_params + n_outs))
    sharded = jax.jit(
        shard_map(_body, mesh=mesh, in_specs=in_specs, out_specs=out_specs,
                  check_rep=False),
        donate_argnums=donate, keep_unused=True)

    concat_in = [np.concatenate([in_maps[c][nm] for c in range(NCORES)],
                                axis=0)
                 for nm in in_names]
    dev_in = [jax.device_put(
        a, jax.sharding.NamedSharding(mesh, PartitionSpec("core")))
        for a in concat_in]

    def one_run():
        zeros = [jax.device_put(
            np.zeros((NCORES * z.shape[0], *z.shape[1:]), z.dtype),
            jax.sharding.NamedSharding(mesh, PartitionSpec("core")))
            for z in zero_outs]
        t0 = time.perf_counter()
        out = sharded(*dev_in, *zeros)
        jax.block_until_ready(out)
        return time.perf_counter() - t0, out

    one_run()                                    # compile warm-up
    times = []
    out = None
    for _ in range(iters):
        dt, out = one_run()
        times.append(dt)
    out_np = [np.asarray(o) for o in out]
    results = [{nm: out_np[i].reshape(NCORES, *out_avals[i].shape)[c]
                for i, nm in enumerate(out_names)}
               for c in range(NCORES)]
    return _assemble(results), times
